# revision 10
# baseline (speedup 1.0000x reference)
"""Trainium2 Bass kernel for nn_CouplingHead (sparse_attention).

Math (per batch element b):
    fc      = concat([f_b, log_eps_b * ones], -1)          # (N, 257)
    q       = fc @ W_q ; k = fc @ W_k                      # (N, 64)
    logits  = q @ k.T / 8                                  # (N, N)   n=query, m=key
    col_sm  = softmax(logits, axis=0)                      # softmax over the QUERY axis n
    pi      = col_sm / N
    y       = N * pi.T @ x                                 # (N, 3)

Sharding: data-parallel over batch B=8, one batch element per NeuronCore.

Per-core kernel strategy (all in the natural [n, m] layout):
  - host passes f_b pre-transposed (fT: feat x n) so no on-chip transposes
  - qT/kT = W.T @ fT via PE float32r matmuls.  fp32r ISA restrictions
    require the output to span all 128 partitions, so the host duplicates
    the weights ([W | W], 257 x 128) and the projection produces qT twice
    (rows 0:64 and 64:128); log_eps enters as a per-partition activation
    bias (bias vector = le * W[256], computed on DVE from a partition-
    broadcast DMA of le)
  - logits blocks [128 n x 1024 m] -> PSUM, ScalarE exp (scale=1/8 fused,
    no max subtraction: logits ~ N(0,1) so exp is safe in fp32)
  - one fused [ones | x | 0-pad] matmul per block accumulates BOTH the
    softmax denominator S[m] (row 0) and the unnormalized y (rows 1:4)
    in PSUM (lhsT zero-padded to 128 cols for the fp32r ISA rule;
    matmul cost is column-bound so the padding is free)
  - 1/S broadcast across partitions via a K=1 ones matmul (plain fp32),
    VectorE multiply E * (1/(S*N)) -> pi, DMA out per block
  - y.T = Ey * (1/S); returned transposed, host untransposes.
"""

import os
import sys

sys.path.insert(0, "/opt/trn_rl_repo")

import numpy as np

import concourse.bass as bass
import concourse.mybir as mybir
import concourse.tile as tile
from concourse import bacc
from concourse.bass import ts, ds

N = 2048
F = 256
H = 64
B = 8
MC = 1024          # m-chunk width (2 chunks)
NBLK = 16          # n blocks of 128
F32 = mybir.dt.float32
F32R = mybir.dt.float32r
AF = mybir.ActivationFunctionType

_CACHE = {}


def build_nc():
    nc = bacc.Bacc("TRN2", target_bir_lowering=False, num_devices=B)
    ft_d = nc.declare_dram_parameter("ft", [F, N], F32R, isOutput=False)
    ox_d = nc.declare_dram_parameter("ox", [128, NBLK * 128], F32R, isOutput=False)
    le_d = nc.declare_dram_parameter("le", [1, 1], F32, isOutput=False)
    wq2_d = nc.declare_dram_parameter("wq2", [F, 128], F32R, isOutput=False)
    wk2_d = nc.declare_dram_parameter("wk2", [F, 128], F32R, isOutput=False)
    wql_d = nc.declare_dram_parameter("wql", [128, 1], F32, isOutput=False)
    wkl_d = nc.declare_dram_parameter("wkl", [128, 1], F32, isOutput=False)
    pi_d = nc.declare_dram_parameter("pi", [N, N], F32, isOutput=True)
    yt_d = nc.declare_dram_parameter("yt", [3, N], F32, isOutput=True)

    with tile.TileContext(nc) as tc:
        with (
            tc.tile_pool(name="singles", bufs=1) as singles,
            tc.tile_pool(name="stat", bufs=2) as stat,
            tc.tile_pool(name="epool", bufs=20) as epool,
            tc.tile_pool(name="pipool", bufs=4) as pipool,
            tc.tile_pool(name="lgps", bufs=2, space="PSUM") as lgps,
            tc.tile_pool(name="sumps", bufs=1, space="PSUM") as sumps,
            tc.tile_pool(name="miscps", bufs=1, space="PSUM") as miscps,
        ):
            # ---- load inputs
            ft_sb = singles.tile([128, 2, N], F32R)
            nc.sync.dma_start(out=ft_sb[:, 0, :], in_=ft_d[0:128, :])
            nc.sync.dma_start(out=ft_sb[:, 1, :], in_=ft_d[128:256, :])
            wq2_sb = singles.tile([128, 2, 128], F32R, tag="wq2")
            wk2_sb = singles.tile([128, 2, 128], F32R, tag="wk2")
            for w_sb, w_d in ((wq2_sb, wq2_d), (wk2_sb, wk2_d)):
                nc.sync.dma_start(out=w_sb[:, 0, :], in_=w_d[0:128, :])
                nc.sync.dma_start(out=w_sb[:, 1, :], in_=w_d[128:256, :])
            wql_sb = singles.tile([128, 1], F32, tag="wql")
            wkl_sb = singles.tile([128, 1], F32, tag="wkl")
            nc.sync.dma_start(out=wql_sb, in_=wql_d[:, :])
            nc.sync.dma_start(out=wkl_sb, in_=wkl_d[:, :])

            # log_eps broadcast to all 128 partitions (DMA partition-step 0)
            le_bc = singles.tile([128, 1], F32, tag="le")
            le_ap = le_d[:, :]
            le_bcast_src = bass.AP(
                tensor=le_ap.tensor, offset=le_ap.offset, ap=[[0, 128], [1, 1]]
            )
            nc.gpsimd.dma_start(out=le_bc, in_=le_bcast_src)

            # ox: per n-block lhsT [128, 128] = [ones | x_block | zeros]
            ox_sb = singles.tile([128, NBLK, 128], F32R, tag="ox")
            nc.sync.dma_start(
                out=ox_sb, in_=ox_d[:, :].rearrange("p (c w) -> p c w", c=NBLK)
            )

            ones_row = singles.tile([1, 128], F32, tag="ones")
            nc.vector.memset(ones_row, 1.0)
            yt_sb = singles.tile([3, N], F32, tag="yt")

            # ---- projections: qq2/kk2 [128, N]; rows 0:64 and 64:128 both
            # hold the projected qT/kT (weights duplicated host-side)
            qq2_sb = singles.tile([128, N], F32R, tag="qq2")
            kk2_sb = singles.tile([128, N], F32R, tag="kk2")
            for w_sb, wl_sb, o_sb in (
                (wq2_sb, wql_sb, qq2_sb),
                (wk2_sb, wkl_sb, kk2_sb),
            ):
                b_sb = stat.tile([128, 1], F32, tag="bias")
                nc.vector.tensor_mul(b_sb, wl_sb, le_bc)
                for half in range(2):
                    ps = lgps.tile([128, 2, 512], F32, tag="lg")
                    for j in range(2):
                        nch = half * 2 + j
                        nc.tensor.matmul(
                            ps[:, j, :],
                            lhsT=w_sb[:, 0, :],
                            rhs=ft_sb[:, 0, ts(nch, 512)],
                            start=True,
                            stop=False,
                        )
                        nc.tensor.matmul(
                            ps[:, j, :],
                            lhsT=w_sb[:, 1, :],
                            rhs=ft_sb[:, 1, ts(nch, 512)],
                            start=False,
                            stop=True,
                        )
                    nc.scalar.activation(
                        o_sb[:, ts(half, 1024)].rearrange("p (a b) -> p a b", a=2),
                        ps,
                        AF.Identity,
                        bias=b_sb,
                    )

            # ---- main: 2 m-chunks of 1024
            for c in range(2):
                mlo = c * MC
                sums = sumps.tile([128, MC], F32, tag="sums")
                e_tiles = []
                for blk in range(NBLK):
                    lg = lgps.tile([128, MC], F32, tag="lg")
                    for j in range(2):
                        nc.tensor.matmul(
                            lg[:, ts(j, 512)],
                            lhsT=qq2_sb[0:64, ds(blk * 128, 128)],
                            rhs=kk2_sb[0:64, ds(mlo + j * 512, 512)],
                            start=True,
                            stop=True,
                        )
                    e = epool.tile([128, MC], F32R, tag="E")
                    nc.scalar.activation(e, lg, AF.Exp, scale=0.125)
                    for j in range(2):
                        nc.tensor.matmul(
                            sums[:, ts(j, 512)],
                            lhsT=ox_sb[:, blk, :],
                            rhs=e[:, ts(j, 512)],
                            start=(blk == 0),
                            stop=(blk == NBLK - 1),
                        )
                    e_tiles.append(e)

                # ---- stats: R1 = 1/(S*N) broadcast over partitions
                sums_sb = stat.tile([4, MC], F32, tag="sums_sb")
                nc.scalar.copy(sums_sb, sums[0:4, :])
                r2 = stat.tile([1, MC], F32, tag="r2")
                nc.vector.reciprocal(r2, sums_sb[0:1, :])
                r1 = stat.tile([1, MC], F32, tag="r1")
                nc.vector.tensor_scalar_mul(r1, r2, 1.0 / float(N))
                rb_ps = miscps.tile([128, MC], F32, tag="rb")
                for j in range(2):
                    nc.tensor.matmul(
                        rb_ps[:, ts(j, 512)],
                        lhsT=ones_row,
                        rhs=r1[:, ts(j, 512)],
                        start=True,
                        stop=True,
                    )
                # y.T slice = Ey * (1/S) = Ey * (R1 * N)
                # (compute engines can't address a partition window starting
                # at 1, so rebase Ey rows 1:4 -> 0:3 with a tiny DMA first)
                ey_sb = stat.tile([3, MC], F32, tag="ey")
                nc.sync.dma_start(out=ey_sb, in_=sums_sb[1:4, :])
                yq = stat.tile([3, MC], F32, tag="yq")
                nc.vector.tensor_mul(yq, ey_sb, rb_ps[0:3, :])
                nc.scalar.mul(yt_sb[:, ds(mlo, MC)], yq, float(N))

                # ---- phase 2: pi = E * R1_bcast, DMA out
                for blk in range(NBLK):
                    pi_t = pipool.tile([128, MC], F32, tag="pi")
                    nc.vector.tensor_mul(pi_t, e_tiles[blk].bitcast(F32), rb_ps)
                    nc.sync.dma_start(
                        out=pi_d[ds(blk * 128, 128), ds(mlo, MC)], in_=pi_t
                    )

            nc.sync.dma_start(out=yt_d[:, :], in_=yt_sb)

    nc.compile()
    return nc


def shard_inputs(f, x_tilde, log_eps, W_q, W_k):
    """Build the per-core input maps (host-side sharding + packing)."""
    W_q = np.asarray(W_q, dtype=np.float32)
    W_k = np.asarray(W_k, dtype=np.float32)
    wq2 = np.ascontiguousarray(np.concatenate([W_q[:F], W_q[:F]], axis=1))
    wk2 = np.ascontiguousarray(np.concatenate([W_k[:F], W_k[:F]], axis=1))
    wql = np.ascontiguousarray(
        np.concatenate([W_q[F], W_q[F]]).reshape(128, 1)
    )
    wkl = np.ascontiguousarray(
        np.concatenate([W_k[F], W_k[F]]).reshape(128, 1)
    )
    in_maps = []
    for b in range(np.asarray(f).shape[0]):
        x_b = np.asarray(x_tilde[b], dtype=np.float32)
        ox = np.zeros((128, NBLK, 128), dtype=np.float32)
        ox[:, :, 0] = 1.0
        ox[:, :, 1:4] = x_b.reshape(NBLK, 128, 3).transpose(1, 0, 2)
        in_maps.append(
            {
                "ft": np.ascontiguousarray(f[b].T.astype(np.float32)),
                "ox": np.ascontiguousarray(ox.reshape(128, NBLK * 128)),
                "le": np.array([[log_eps[b]]], dtype=np.float32),
                "wq2": wq2,
                "wk2": wk2,
                "wql": wql,
                "wkl": wkl,
            }
        )
    return in_maps


def kernel(f, x_tilde, log_eps, W_q, W_k):
    from concourse.bass_utils import run_bass_kernel_spmd

    assert f.shape == (B, N, F)
    if "nc" not in _CACHE:
        _CACHE["nc"] = build_nc()
    nc = _CACHE["nc"]

    in_maps = shard_inputs(f, x_tilde, log_eps, W_q, W_k)
    trace = bool(os.environ.get("KBENCH_TRACE"))
    res = run_bass_kernel_spmd(nc, in_maps, list(range(B)), trace=trace)
    _CACHE["last"] = res

    pi = np.stack([res.results[b]["pi"] for b in range(B)])
    y = np.stack([np.ascontiguousarray(res.results[b]["yt"].T) for b in range(B)])
    return y, pi
